# revision 1
# baseline (speedup 1.0000x reference)
"""Multi-head attention (B=4, S=2048, D=1024, H=16) on 8 Trainium2 cores.

Sharding (v5): core c -> head-pair p = c (2 heads, 128 output dims), all 4
batches.  This makes the valid_len truncation SPMD-uniform: every core runs
the same per-batch k-loop trip counts kc_b = ceil(valid_len[b]/128), so the
~50% of attention work beyond the key-padding boundary is simply never
emitted.  W_o is row-split by head-pair; each core emits a full-shape
[B, S, D] fp16 partial and the host sums the 8 partials.

Masking: the host zeroes xv columns at k >= valid_len[b] and supplies a
masked ones-column, so masked keys contribute exactly 0 to both the
attention*V accumulation and the softmax denominator.  exp then needs no
bias at all (scores at masked-but-computed boundary columns are finite).

Device layout notes:
  - matmul computes lhsT.T @ rhs with contraction on the partition dim.
  - Q/K are produced transposed ([dout, s]) so scores come out as
    scores^T [k, q]; 1/sqrt(hd) folds into the ScalarE exp op's scale.
  - V is natural [s, dout] with a (host-masked) ones column per head, so the
    attention*V matmul also emits the softmax denominators.
  - All matmul inputs bf16 (4x faster PE than fp32), fp32 PSUM accumulation.
  - PSUM budget: scp 2x2 banks + av 2 + projection 2 = 8; Q/K projection of
    batch b+1 is emitted ahead of attention of batch b so the PE fills the
    gaps of the ACT-bound attention pipeline.
"""

import contextlib

import numpy as np
import ml_dtypes

import concourse.bacc as bacc
import concourse.mybir as mybir
import concourse.tile as tile
from concourse.bass_utils import run_bass_kernel_spmd

BF16 = mybir.dt.bfloat16
F16 = mybir.dt.float16
F32 = mybir.dt.float32
AF = mybir.ActivationFunctionType

B, S, D, H, HD = 4, 2048, 1024, 16, 64
NQB = S // 512        # query blocks of 512

_cache = {}


def _qk_proj_b(nc, b, kcb, ap, stream, psum, wq_sb, wk_sb, qT_sb, kT_sb):
    """Q/K projection for one batch (both heads of the pair = 128 dims)."""
    for qb in range(NQB):
        psq = psum.tile([128, 512], F32, tag="pqk", name="psq")
        for dj in range(8):
            xqt = stream.tile([128, 512], BF16, tag="xq", name="xqt")
            nc.sync.dma_start(
                xqt[:],
                ap[f"xq{b}"][dj * 128:(dj + 1) * 128,
                             qb * 512:(qb + 1) * 512])
            nc.tensor.matmul(psq[:], wq_sb[dj][:], xqt[:],
                             start=(dj == 0), stop=(dj == 7))
        nc.vector.tensor_copy(qT_sb[b][:, qb * 512:(qb + 1) * 512], psq[:])
    nk = kcb * 128
    for kb in range((nk + 511) // 512):
        n = min(512, nk - kb * 512)
        psk = psum.tile([128, 512], F32, tag="pqk", name="psk")
        for dj in range(8):
            xkt = stream.tile([128, 512], BF16, tag="xk", name="xkt")
            nc.sync.dma_start(
                xkt[:, 0:n],
                ap[f"xk{b}"][dj * 128:(dj + 1) * 128,
                             kb * 512:kb * 512 + n])
            nc.tensor.matmul(psk[:, 0:n], wk_sb[dj][:], xkt[:, 0:n],
                             start=(dj == 0), stop=(dj == 7))
        nc.vector.tensor_copy(
            kT_sb[b][:, kb * 512:kb * 512 + n], psk[:, 0:n])


def _v_proj_b(nc, b, kcb, ap, stream, psum, wv_sb, v_sb, vm_sb):
    """V projection for one batch, natural [s, dout] + host-masked ones
    column per head.  v_sb[b][sc] is [128, 2, HD+1]."""
    xv_sb = []
    for dj in range(8):
        xvt = stream.tile([128, 2048], BF16, tag="xv", name="xvt", bufs=9)
        nc.sync.dma_start(
            xvt[:, 0:kcb * 128],
            ap[f"xv{b}"][dj * 128:(dj + 1) * 128, :])
        xv_sb.append(xvt)
    for sc in range(kcb):
        psv = psum.tile([128, 2, HD], F32, tag="pv", name="psv")
        for dj in range(8):
            nc.tensor.matmul(
                psv[:], xv_sb[dj][:, sc * 128:(sc + 1) * 128],
                wv_sb[dj][:], start=(dj == 0), stop=(dj == 7))
        nc.vector.tensor_copy(v_sb[b][sc][:, :, 0:HD], psv[:])
        nc.vector.tensor_copy(
            v_sb[b][sc][:, :, HD], vm_sb[:, b * 16 + sc, :])


def _attention_b(nc, b, kcb, psum, expool, wrk, qT_sb, kT_sb, v_sb, ctx_sb):
    """Attention for one batch (2 heads): ACT-bound pipeline over (qb, kc)."""
    for qb in range(4):
        av = psum.tile([HD + 1, 2, 512], F32, tag="av", name="av")
        for kc in range(kcb):
            scp = psum.tile([128, 2, 512], F32, tag="sc", name="scp", bufs=2)
            for h2 in range(2):
                nc.tensor.matmul(
                    scp[:, h2, :],
                    kT_sb[b][64 * h2:64 * h2 + 64, kc * 128:(kc + 1) * 128],
                    qT_sb[b][64 * h2:64 * h2 + 64, qb * 512:(qb + 1) * 512],
                    start=True, stop=True)
            ex = expool.tile([128, 2, 512], BF16, tag="ex", name="ex")
            nc.scalar.activation(ex[:], scp[:], AF.Exp, scale=0.125)
            for h2 in range(2):
                nc.tensor.matmul(
                    av[:, h2, :], v_sb[b][kc][:, h2, :], ex[:, h2, :],
                    start=(kc == 0), stop=(kc == kcb - 1))
        # Copy av to SBUF (frees the PSUM slot), then normalize off the
        # critical path: ctx[m, q] = av[m, q] / av[64, q].
        avc = wrk.tile([HD + 1, 2, 512], F32, tag="avc", name="avc")
        nc.vector.tensor_copy(avc[:, 0, :], av[:, 0, :])
        nc.scalar.activation(avc[:, 1, :], av[:, 1, :], AF.Copy)
        # Broadcast the sums row to partitions 0..63, then approx-recip there
        # (reciprocal_approx_fast breaks on 1-partition APs, and
        # partition_broadcast only reads partition 0 of its source).
        r0 = wrk.tile([1, 1024], F32, tag="r0", name="r0")
        nc.sync.dma_start(r0[:], avc[HD:HD + 1, :, :])
        bc = wrk.tile([HD, 1024], F32, tag="bc", name="bc")
        nc.gpsimd.partition_broadcast(bc[:], r0[0:1, :])
        recb = wrk.tile([HD, 1024], F32, tag="recb", name="recb")
        nc.vector.reciprocal_approx_fast(recb[:], bc[:])
        nc.vector.tensor_mul(
            ctx_sb[b][0:HD, qb * 512:(qb + 1) * 512],
            avc[0:HD, 0, :], recb[:, 0:512])
        tmp = wrk.tile([HD, 512], BF16, tag="tmpb", name="tmp")
        nc.vector.tensor_mul(tmp[:], avc[0:HD, 1, :], recb[:, 512:1024])
        nc.sync.dma_start(
            ctx_sb[b][HD:128, qb * 512:(qb + 1) * 512], tmp[:])


def _o_proj_b(nc, b, ap, psum, wrk, ctx_sb, wo_sb):
    """Output projection partial for one batch; alternates the two fill-pool
    PSUM slots (pqk/pv) so it pipelines 2-deep."""
    for sc in range(16):
        for ih in range(2):
            tg = "pqk" if (2 * sc + ih) % 2 == 0 else "pv"
            po = psum.tile([128, 512], F32, tag=tg, name="po")
            nc.tensor.matmul(
                po[:], ctx_sb[b][:, sc * 128:(sc + 1) * 128],
                wo_sb[0][:, ih * 512:(ih + 1) * 512],
                start=True, stop=True)
            ot = wrk.tile([128, 512], F16, tag="ot", name="ot", bufs=4)
            if ih == 0:
                nc.vector.tensor_copy(ot[:], po[:])
            else:
                nc.scalar.activation(ot[:], po[:], AF.Copy)
            nc.sync.dma_start(
                ap["out"][b, sc * 128:(sc + 1) * 128,
                          ih * 512:(ih + 1) * 512], ot[:])


def _emit(nc, tc, ap, kcs):
    es = contextlib.ExitStack()
    with es:
        const = es.enter_context(tc.tile_pool(name="const", bufs=1))
        resid = es.enter_context(tc.tile_pool(name="resid", bufs=1))
        stream = es.enter_context(tc.tile_pool(name="stream", bufs=3))
        expool = es.enter_context(tc.tile_pool(name="expool", bufs=3))
        wrk = es.enter_context(tc.tile_pool(name="wrk", bufs=2))

        # constants: per-dj [din-chunk, dout=128] weight tiles for the pair
        wq_sb = [const.tile([128, 128], BF16, tag=f"wq{i}", name=f"wq{i}")
                 for i in range(8)]
        wk_sb = [const.tile([128, 128], BF16, tag=f"wk{i}", name=f"wk{i}")
                 for i in range(8)]
        wv_sb = [const.tile([128, 2, HD], BF16, tag=f"wv{i}", name=f"wv{i}")
                 for i in range(8)]
        wo_sb = [const.tile([128, D], BF16, tag="wo", name="wo")]
        vm_sb = const.tile([128, 64, 2], BF16, tag="vmask", name="vmask")
        nc.sync.dma_start(vm_sb[:], ap["vones"])
        for i in range(8):
            nc.sync.dma_start(wq_sb[i][:], ap["wq"][i * 128:(i + 1) * 128, :])
            nc.sync.dma_start(wk_sb[i][:], ap["wk"][i * 128:(i + 1) * 128, :])
            nc.sync.dma_start(wv_sb[i][:],
                              ap["wv"][i * 128:(i + 1) * 128, :, :])
        nc.sync.dma_start(wo_sb[0][:], ap["wo"])

        # residents (per batch)
        qT_sb = [resid.tile([128, S], BF16, tag=f"qT{b}", name=f"qT{b}")
                 for b in range(B)]
        kT_sb = [resid.tile([128, kcs[b] * 128], BF16, tag=f"kT{b}",
                            name=f"kT{b}") for b in range(B)]
        ctx_sb = [resid.tile([128, S], BF16, tag=f"ctx{b}", name=f"ctx{b}")
                  for b in range(B)]
        v_sb = [[resid.tile([128, 2, HD + 1], BF16, tag=f"v{b}_{i}",
                            name=f"v{b}_{i}") for i in range(kcs[b])]
                for b in range(B)]

        # Fill pool (2 banks: pqk + pv) carries Q/K/V projections and the
        # O-projection; they run in the PE gaps of the ACT-bound attention.
        order = sorted(range(B), key=lambda b: -kcs[b])
        with tc.tile_pool(name="fill_psum", bufs=1, space="PSUM") as fill:
            b0 = order[0]
            _qk_proj_b(nc, b0, kcs[b0], ap, stream, fill,
                       wq_sb, wk_sb, qT_sb, kT_sb)
            _v_proj_b(nc, b0, kcs[b0], ap, stream, fill, wv_sb, v_sb, vm_sb)
            with tc.tile_pool(name="at_psum", bufs=1, space="PSUM") as at_psum:
                # attention first = higher scheduler priority; projections and
                # O fill the PE gaps of the ACT-bound pipeline.
                for i, b in enumerate(order):
                    _attention_b(nc, b, kcs[b], at_psum, expool, wrk,
                                 qT_sb, kT_sb, v_sb, ctx_sb)
                    if i + 1 < B:
                        nb = order[i + 1]
                        _qk_proj_b(nc, nb, kcs[nb], ap, stream, fill,
                                   wq_sb, wk_sb, qT_sb, kT_sb)
                        _v_proj_b(nc, nb, kcs[nb], ap, stream, fill,
                                  wv_sb, v_sb, vm_sb)
                    _o_proj_b(nc, b, ap, fill, wrk, ctx_sb, wo_sb)


def _build(kcs):
    key = ("nc", tuple(kcs))
    if key in _cache:
        return _cache[key]
    nc = bacc.Bacc("TRN2", target_bir_lowering=False, debug=False, num_devices=8)
    ap = {"wq": nc.dram_tensor("wq", [D, 128], BF16, kind="ExternalInput").ap(),
          "wk": nc.dram_tensor("wk", [D, 128], BF16, kind="ExternalInput").ap(),
          "wv": nc.dram_tensor("wv", [D, 2, HD], BF16, kind="ExternalInput").ap(),
          "wo": nc.dram_tensor("wo", [128, D], BF16, kind="ExternalInput").ap(),
          "vones": nc.dram_tensor("vones", [128, 64, 2], BF16,
                                  kind="ExternalInput").ap(),
          "out": nc.dram_tensor("out", [B, S, D], F16,
                                kind="ExternalOutput").ap()}
    for b in range(B):
        ap[f"xq{b}"] = nc.dram_tensor(f"xq{b}", [D, S], BF16,
                                      kind="ExternalInput").ap()
        ap[f"xk{b}"] = nc.dram_tensor(f"xk{b}", [D, kcs[b] * 128], BF16,
                                      kind="ExternalInput").ap()
        ap[f"xv{b}"] = nc.dram_tensor(f"xv{b}", [D, kcs[b] * 128], BF16,
                                      kind="ExternalInput").ap()
    with tile.TileContext(nc) as tc:
        _emit(nc, tc, ap, kcs)
    nc.compile()
    _cache[key] = nc
    return nc


def _in_maps(kcs, queries, keys, values, valid_len, W_q, W_k, W_v, W_o):
    bf = ml_dtypes.bfloat16
    # host-masked ones column: 1 where k < valid_len[b], else 0
    # vones[p, b*16+sc, h] = 1 if sc*128+p < valid_len[b] else 0
    kpos = np.arange(16 * 128).reshape(16, 128)
    vones = np.zeros((128, 64, 2), bf)
    for b in range(B):
        v1 = (kpos < int(valid_len[b])).astype(bf)  # [16, 128]
        vones[:, b * 16:(b + 1) * 16, :] = v1.T[:, :, None]
    maps = []
    for c in range(8):
        j0 = 128 * c
        m = {
            "wq": np.ascontiguousarray(W_q[j0:j0 + 128, :].T).astype(bf),
            "wk": np.ascontiguousarray(W_k[j0:j0 + 128, :].T).astype(bf),
            "wv": np.ascontiguousarray(
                W_v[j0:j0 + 128, :].T).astype(bf).reshape(D, 2, HD),
            "wo": np.ascontiguousarray(W_o[:, j0:j0 + 128].T).astype(bf),
            "vones": vones,
        }
        for b in range(B):
            nk = kcs[b] * 128
            xv = values[b][:nk].T.copy()      # [D, nk]
            xv[:, int(valid_len[b]):] = 0.0   # mask padding rows of v
            m[f"xq{b}"] = np.ascontiguousarray(queries[b].T).astype(bf)
            m[f"xk{b}"] = np.ascontiguousarray(keys[b][:nk].T).astype(bf)
            m[f"xv{b}"] = xv.astype(bf)
        maps.append(m)
    return maps


def kernel(queries, keys, values, valid_len, W_q, W_k, W_v, W_o, _run_kwargs=None):
    queries = np.asarray(queries, np.float32)
    keys = np.asarray(keys, np.float32)
    values = np.asarray(values, np.float32)
    valid_len = np.asarray(valid_len)
    W_q = np.asarray(W_q, np.float32)
    W_k = np.asarray(W_k, np.float32)
    W_v = np.asarray(W_v, np.float32)
    W_o = np.asarray(W_o, np.float32)

    kcs = [max(1, min(16, -(-int(valid_len[b]) // 128))) for b in range(B)]
    nc = _build(kcs)
    maps = _in_maps(kcs, queries, keys, values, valid_len, W_q, W_k, W_v, W_o)
    res = run_bass_kernel_spmd(nc, maps, list(range(8)), **(_run_kwargs or {}))
    out = np.zeros((B, S, D), np.float32)
    for c in range(8):
        out += res.results[c]["out"].astype(np.float32)
    if _run_kwargs:
        _cache["last_results"] = res
    return out



# revision 2
# speedup vs baseline: 1.2162x; 1.2162x over previous
"""Multi-head attention (B=4, S=2048, D=1024, H=16) on 8 Trainium2 cores.

Sharding (v6): core c -> head-pair p = c (2 heads, 128 output dims), all 4
batches.  Every core runs the same per-batch k-loop trip counts
kc_b = ceil(valid_len[b]/128), so the key-padding truncation is SPMD-uniform.
W_o is row-split by head-pair; each core emits a full-shape [B, S, D] fp16
partial and the host sums the 8 partials.

v6 changes vs v5 (419us):
  - Softmax denominator via a second col-tiled matmul pair: lhsT = masked-ones
    [128, 64] -> den replicated on the same partitions as the head dims
    (avden[:, 1, :]).  Normalization becomes lane-aligned: ONE
    reciprocal_approx_fast + ONE tensor_mul (PSUM read) per (b, qb) --
    no r0 DMA, no gpsimd partition_broadcast, no tmp DMA, no avc copies.
  - AV matmuls col-tiled (M=64 at col groups 0/64) so the two heads run
    concurrently in the PE array; same for the den pair.
  - ScalarE runs ONLY the exp activations; all PSUM->SBUF drains on DVE.
  - Inputs staged in [128, 8, S] blocked layout -> one DMA per 1024-col half
    instead of 8 per 512 block (sync-engine dispatch was 260us in v5).
  - O-projection output staged [128, 2, 512] f16, one DMA per (b, sc).

Device layout notes:
  - matmul computes lhsT.T @ rhs, contraction on the partition dim.
  - Q/K are produced transposed ([dout, s]); scores come out [k, q] per head
    in scp[:, h, :]; 1/sqrt(hd) folds into the exp op's scale.
  - V is natural [s, 2, 64]; masked-ones lhsT gives den rows.
  - All matmul inputs bf16, fp32 PSUM.
  - PSUM: sc 2x2 banks + avden 2 + pqk 1 + pv 1 = 8 banks.
"""

import contextlib

import numpy as np
import ml_dtypes

import concourse.bacc as bacc
import concourse.mybir as mybir
import concourse.tile as tile
from concourse.bass_utils import run_bass_kernel_spmd

BF16 = mybir.dt.bfloat16
F16 = mybir.dt.float16
F32 = mybir.dt.float32
AF = mybir.ActivationFunctionType

B, S, D, H, HD = 4, 2048, 1024, 16, 64

_cache = {}


def _qk_proj_b(nc, b, kcb, ap, stream, psum, wq_sb, wk_sb, qT_sb, kT_sb):
    """Q/K projection for one batch (both heads of the pair = 128 dims)."""
    for hf in range(2):
        xqt = stream.tile([128, 8, 1024], BF16, tag="xq", name="xqt")
        nc.sync.dma_start(xqt[:], ap[f"xq{b}"][:, :, hf * 1024:(hf + 1) * 1024])
        for q2 in range(2):
            qb = hf * 2 + q2
            psq = psum.tile([128, 512], F32, tag="pqk", bufs=1, name="psq")
            for dj in range(8):
                nc.tensor.matmul(psq[:], wq_sb[:, dj, :],
                                 xqt[:, dj, q2 * 512:(q2 + 1) * 512],
                                 start=(dj == 0), stop=(dj == 7))
            nc.vector.tensor_copy(qT_sb[b][:, qb * 512:(qb + 1) * 512], psq[:])
    nk = kcb * 128
    for hf in range((nk + 1023) // 1024):
        n = min(1024, nk - hf * 1024)
        xkt = stream.tile([128, 8, 1024], BF16, tag="xk", name="xkt")
        nc.sync.dma_start(xkt[:, :, 0:n],
                          ap[f"xk{b}"][:, :, hf * 1024:hf * 1024 + n])
        for k2 in range((n + 511) // 512):
            m = min(512, n - k2 * 512)
            psk = psum.tile([128, 512], F32, tag="pqk", bufs=1, name="psk")
            for dj in range(8):
                nc.tensor.matmul(psk[:, 0:m], wk_sb[:, dj, :],
                                 xkt[:, dj, k2 * 512:k2 * 512 + m],
                                 start=(dj == 0), stop=(dj == 7))
            o = hf * 1024 + k2 * 512
            nc.vector.tensor_copy(kT_sb[b][:, o:o + m], psk[:, 0:m])


def _v_proj_b(nc, b, kcb, ap, stream, psum, wv_sb, v_sb):
    """V projection for one batch, natural [s, 2, 64] per 128-key chunk."""
    nk = kcb * 128
    for hf in range((nk + 1023) // 1024):
        n = min(1024, nk - hf * 1024)
        xvt = stream.tile([128, 8, 1024], BF16, tag="xv", name="xvt")
        nc.sync.dma_start(xvt[:, :, 0:n],
                          ap[f"xv{b}"][:, :, hf * 1024:hf * 1024 + n])
        for s2 in range(n // 128):
            sc = hf * 8 + s2
            psv = psum.tile([128, 2, HD], F32, tag="pv", bufs=1, name="psv")
            for dj in range(8):
                nc.tensor.matmul(psv[:], xvt[:, dj, s2 * 128:(s2 + 1) * 128],
                                 wv_sb[:, dj, :, :],
                                 start=(dj == 0), stop=(dj == 7))
            nc.vector.tensor_copy(v_sb[b][sc][:], psv[:])


def _attention_b(nc, b, kcb, psum, expool, wrk, qT_sb, kT_sb, v_sb, ones_sb,
                 ctx_sb):
    """Attention for one batch (2 heads): ACT-bound pipeline over (qb, kc).

    avden[:, 0, :]: rows 0-63 = head0 AV dims, 64-127 = head1 AV dims
    (col-tiled concurrent matmul pair).  avden[:, 1, :]: same layout but
    denominators replicated 64x from masked-ones lhsT -> normalization is
    a single lane-aligned recip + mul."""
    qT, kT, ctx = qT_sb[b], kT_sb[b], ctx_sb[b]
    for qb in range(4):
        avden = psum.tile([128, 2, 512], F32, tag="avden", bufs=1, name="avden")
        for kc in range(kcb):
            scp = psum.tile([128, 2, 512], F32, tag="sc", bufs=2, name="scp")
            for h in range(2):
                nc.tensor.matmul(
                    scp[:, h, :],
                    kT[64 * h:64 * h + 64, kc * 128:(kc + 1) * 128],
                    qT[64 * h:64 * h + 64, qb * 512:(qb + 1) * 512],
                    start=True, stop=True)
            ex = expool.tile([128, 2, 512], BF16, tag="ex", name="ex")
            nc.scalar.activation(ex[:], scp[:], AF.Exp, scale=0.125)
            first, last = kc == 0, kc == kcb - 1
            ones = ones_sb[:, 1 + b, :] if last else ones_sb[:, 0, :]
            for h in range(2):
                nc.tensor.matmul(
                    avden[64 * h:64 * h + 64, 0, :], v_sb[b][kc][:, h, :],
                    ex[:, h, :], start=first, stop=last)
            for h in range(2):
                nc.tensor.matmul(
                    avden[64 * h:64 * h + 64, 1, :], ones,
                    ex[:, h, :], start=first, stop=last)
        recb = wrk.tile([128, 512], F32, tag="recb", name="recb")
        nc.vector.reciprocal_approx_fast(recb[:], avden[:, 1, :])
        nc.vector.tensor_mul(ctx[:, qb * 512:(qb + 1) * 512],
                             avden[:, 0, :], recb[:])


def _o_proj_b(nc, b, ap, psum, wrk, ctx_sb, wo_sb):
    """Output projection partial for one batch; alternates pqk/pv banks."""
    for sc in range(16):
        ot = wrk.tile([128, 2, 512], F16, tag="ot", bufs=4, name="ot")
        for ih in range(2):
            tg = "pqk" if ih == 0 else "pv"
            po = psum.tile([128, 512], F32, tag=tg, bufs=1, name="po")
            nc.tensor.matmul(po[:], ctx_sb[b][:, sc * 128:(sc + 1) * 128],
                             wo_sb[:, ih * 512:(ih + 1) * 512],
                             start=True, stop=True)
            nc.vector.tensor_copy(ot[:, ih, :], po[:])
        nc.sync.dma_start(ap["out"][b, sc * 128:(sc + 1) * 128, :], ot[:])


def _emit(nc, tc, ap, kcs):
    es = contextlib.ExitStack()
    with es:
        const = es.enter_context(tc.tile_pool(name="const", bufs=1))
        resid = es.enter_context(tc.tile_pool(name="resid", bufs=1))
        stream = es.enter_context(tc.tile_pool(name="stream", bufs=2))
        expool = es.enter_context(tc.tile_pool(name="expool", bufs=3))
        wrk = es.enter_context(tc.tile_pool(name="wrk", bufs=2))

        wq_sb = const.tile([128, 8, 128], BF16, tag="wq", name="wq")
        wk_sb = const.tile([128, 8, 128], BF16, tag="wk", name="wk")
        wv_sb = const.tile([128, 8, 2, HD], BF16, tag="wv", name="wv")
        wo_sb = const.tile([128, D], BF16, tag="wo", name="wo")
        ones_sb = const.tile([128, 5, 64], BF16, tag="ones", name="ones")
        nc.sync.dma_start(wq_sb[:], ap["wq"])
        nc.sync.dma_start(wk_sb[:], ap["wk"])
        nc.sync.dma_start(wv_sb[:], ap["wv"])
        nc.sync.dma_start(wo_sb[:], ap["wo"])
        nc.sync.dma_start(ones_sb[:], ap["ones"])

        qT_sb = [resid.tile([128, S], BF16, tag=f"qT{b}", name=f"qT{b}")
                 for b in range(B)]
        kT_sb = [resid.tile([128, kcs[b] * 128], BF16, tag=f"kT{b}",
                            name=f"kT{b}") for b in range(B)]
        ctx_sb = [resid.tile([128, S], BF16, tag=f"ctx{b}", name=f"ctx{b}")
                  for b in range(B)]
        v_sb = [[resid.tile([128, 2, HD], BF16, tag=f"v{b}_{i}",
                            name=f"v{b}_{i}") for i in range(kcs[b])]
                for b in range(B)]

        order = sorted(range(B), key=lambda b: -kcs[b])
        with tc.tile_pool(name="psum", bufs=1, space="PSUM") as psum:
            b0 = order[0]
            _qk_proj_b(nc, b0, kcs[b0], ap, stream, psum, wq_sb, wk_sb,
                       qT_sb, kT_sb)
            _v_proj_b(nc, b0, kcs[b0], ap, stream, psum, wv_sb, v_sb)
            for i, b in enumerate(order):
                _attention_b(nc, b, kcs[b], psum, expool, wrk, qT_sb, kT_sb,
                             v_sb, ones_sb, ctx_sb)
                if i + 1 < B:
                    nb = order[i + 1]
                    _qk_proj_b(nc, nb, kcs[nb], ap, stream, psum, wq_sb,
                               wk_sb, qT_sb, kT_sb)
                    _v_proj_b(nc, nb, kcs[nb], ap, stream, psum, wv_sb, v_sb)
                _o_proj_b(nc, b, ap, psum, wrk, ctx_sb, wo_sb)


def _build(kcs):
    key = ("nc", tuple(kcs))
    if key in _cache:
        return _cache[key]
    nc = bacc.Bacc("TRN2", target_bir_lowering=False, debug=False,
                   num_devices=8)
    ap = {"wq": nc.dram_tensor("wq", [128, 8, 128], BF16,
                               kind="ExternalInput").ap(),
          "wk": nc.dram_tensor("wk", [128, 8, 128], BF16,
                               kind="ExternalInput").ap(),
          "wv": nc.dram_tensor("wv", [128, 8, 2, HD], BF16,
                               kind="ExternalInput").ap(),
          "wo": nc.dram_tensor("wo", [128, D], BF16,
                               kind="ExternalInput").ap(),
          "ones": nc.dram_tensor("ones", [128, 5, 64], BF16,
                                 kind="ExternalInput").ap(),
          "out": nc.dram_tensor("out", [B, S, D], F16,
                                kind="ExternalOutput").ap()}
    for b in range(B):
        ap[f"xq{b}"] = nc.dram_tensor(f"xq{b}", [128, 8, S], BF16,
                                      kind="ExternalInput").ap()
        ap[f"xk{b}"] = nc.dram_tensor(f"xk{b}", [128, 8, kcs[b] * 128], BF16,
                                      kind="ExternalInput").ap()
        ap[f"xv{b}"] = nc.dram_tensor(f"xv{b}", [128, 8, kcs[b] * 128], BF16,
                                      kind="ExternalInput").ap()
    with tile.TileContext(nc) as tc:
        _emit(nc, tc, ap, kcs)
    nc.compile()
    _cache[key] = nc
    return nc


def _blocked(x2d):
    """[Dsub, N] -> [128, Dsub//128, N] blocked layout (partition, dj, col)."""
    d, n = x2d.shape
    return np.ascontiguousarray(
        x2d.reshape(d // 128, 128, n).transpose(1, 0, 2))


def _in_maps(kcs, queries, keys, values, valid_len, W_q, W_k, W_v, W_o):
    bf = ml_dtypes.bfloat16
    # Shared across cores (data-parallel over the full batch/seq).
    shared = {}
    for b in range(B):
        nk = kcs[b] * 128
        xv = values[b][:nk].T.copy()      # [D, nk]
        xv[:, int(valid_len[b]):] = 0.0   # mask padding rows of v
        shared[f"xq{b}"] = _blocked(queries[b].T.astype(bf))
        shared[f"xk{b}"] = _blocked(keys[b][:nk].T.astype(bf))
        shared[f"xv{b}"] = _blocked(xv.astype(bf))
    ones = np.zeros((128, 5, 64), bf)
    ones[:, 0, :] = 1.0
    p = np.arange(128)
    for b in range(B):
        valid = ((kcs[b] - 1) * 128 + p < int(valid_len[b])).astype(bf)
        ones[:, 1 + b, :] = valid[:, None]
    shared["ones"] = ones

    maps = []
    for c in range(8):
        j0 = 128 * c
        m = dict(shared)
        m["wq"] = _blocked(
            np.ascontiguousarray(W_q[j0:j0 + 128, :].T).astype(bf))
        m["wk"] = _blocked(
            np.ascontiguousarray(W_k[j0:j0 + 128, :].T).astype(bf))
        m["wv"] = _blocked(
            np.ascontiguousarray(W_v[j0:j0 + 128, :].T).astype(bf)
        ).reshape(128, 8, 2, HD)
        m["wo"] = np.ascontiguousarray(W_o[:, j0:j0 + 128].T).astype(bf)
        maps.append(m)
    return maps


def kernel(queries, keys, values, valid_len, W_q, W_k, W_v, W_o,
           _run_kwargs=None):
    queries = np.asarray(queries, np.float32)
    keys = np.asarray(keys, np.float32)
    values = np.asarray(values, np.float32)
    valid_len = np.asarray(valid_len)
    W_q = np.asarray(W_q, np.float32)
    W_k = np.asarray(W_k, np.float32)
    W_v = np.asarray(W_v, np.float32)
    W_o = np.asarray(W_o, np.float32)

    kcs = [max(1, min(16, -(-int(valid_len[b]) // 128))) for b in range(B)]
    nc = _build(kcs)
    maps = _in_maps(kcs, queries, keys, values, valid_len, W_q, W_k, W_v, W_o)
    res = run_bass_kernel_spmd(nc, maps, list(range(8)), **(_run_kwargs or {}))
    out = np.zeros((B, S, D), np.float32)
    for c in range(8):
        out += res.results[c]["out"].astype(np.float32)
    if _run_kwargs:
        _cache["last_results"] = res
    return out


# revision 5
# speedup vs baseline: 1.3244x; 1.0890x over previous
"""Multi-head attention (B=4, S=2048, D=1024, H=16) on 8 Trainium2 cores.

Sharding: core c -> head-pair p = c (2 heads, 128 output dims), all 4
batches.  Every core runs the same per-batch k-loop trip counts
kc_b = ceil(valid_len[b]/128), so the key-padding truncation is SPMD-uniform.
W_o is row-split by head-pair; each core emits a full-shape [B, S, D] fp16
partial and the host sums the 8 partials.

v7: engines execute their instruction queues in emission order, so overlap
must be programmed, not hoped for.  The kernel is one global software-
pipelined stream over attention tiles (b, qb, kc):

    scores(t) -> exp(t) -> [av/den(t-1)] -> [norm when qb done] -> fill

where "fill" pops one closure from a queue holding Q/K/V-projection matmul
groups of the next batch and O-projection chunks of finished query blocks.
This keeps the PE busy during every exp wait and keeps ACT back-to-back.

Math per tile: scores = row-tiled concurrent matmul pair (K=64 at row
groups 0/64); AV = col-tiled concurrent pair (M=64 at col groups 0/64);
denominators via a second col-tiled pair with masked-ones lhsT, landing
den on the same partitions as the AV dims -> normalization is one
lane-aligned reciprocal_approx_fast + tensor_mul from PSUM.

PSUM: sc 2x2 banks + avden 2 + pqk 1 + pv 1 = 8 banks.
"""

import contextlib

import numpy as np
import ml_dtypes

import concourse.bacc as bacc
import concourse.mybir as mybir
import concourse.tile as tile
from concourse.bass_utils import run_bass_kernel_spmd

BF16 = mybir.dt.bfloat16
F16 = mybir.dt.float16
F32 = mybir.dt.float32
AF = mybir.ActivationFunctionType

B, S, D, H, HD = 4, 2048, 1024, 16, 64

_cache = {}


class _Emitter:
    def __init__(self, nc, tc, ap, kcs):
        self.nc = nc
        self.ap = ap
        self.kcs = kcs
        self.fills = []

        es = self.es = contextlib.ExitStack()
        const = es.enter_context(tc.tile_pool(name="const", bufs=1))
        resid = es.enter_context(tc.tile_pool(name="resid", bufs=1))
        self.stream = es.enter_context(tc.tile_pool(name="stream", bufs=2))
        self.expool = es.enter_context(tc.tile_pool(name="expool", bufs=3))
        self.wrk = es.enter_context(tc.tile_pool(name="wrk", bufs=2))
        self.psum = es.enter_context(
            tc.tile_pool(name="psum", bufs=1, space="PSUM"))

        self.wq = const.tile([128, 8, 128], BF16, tag="wq", name="wq")
        self.wk = const.tile([128, 8, 128], BF16, tag="wk", name="wk")
        self.wv = const.tile([128, 8, 2, HD], BF16, tag="wv", name="wv")
        self.wo = const.tile([128, D], BF16, tag="wo", name="wo")
        self.ones = const.tile([128, 5, 64], BF16, tag="ones", name="ones")
        for n, t in [("wq", self.wq), ("wk", self.wk), ("wv", self.wv),
                     ("wo", self.wo), ("ones", self.ones)]:
            nc.sync.dma_start(t[:], ap[n])

        self.qT = [resid.tile([128, S], BF16, tag=f"qT{b}", name=f"qT{b}")
                   for b in range(B)]
        self.kT = [resid.tile([128, kcs[b] * 128], BF16, tag=f"kT{b}",
                              name=f"kT{b}") for b in range(B)]
        self.ctx = [resid.tile([128, S], BF16, tag=f"ctx{b}", name=f"ctx{b}")
                    for b in range(B)]
        self.v = [[resid.tile([128, 2, HD], BF16, tag=f"v{b}_{i}",
                              name=f"v{b}_{i}") for i in range(kcs[b])]
                  for b in range(B)]

    # ---- fill closures (projections of a later batch, O-proj chunks) ----

    def push_proj(self, b):
        """Emit input DMAs for batch b now; queue its matmul groups."""
        nc, ap, kcb = self.nc, self.ap, self.kcs[b]
        nk = kcb * 128
        xqt, xkt, xvt = {}, {}, {}
        for hf in range(2):
            t = self.stream.tile([128, 8, 1024], BF16, tag="xq", bufs=3,
                                 name="xqt")
            nc.sync.dma_start(t[:], ap[f"xq{b}"][:, :, hf * 1024:(hf + 1) * 1024])
            xqt[hf] = t
        for hf in range((nk + 1023) // 1024):
            n = min(1024, nk - hf * 1024)
            t = self.stream.tile([128, 8, 1024], BF16, tag="xk", name="xkt")
            nc.sync.dma_start(t[:, :, 0:n],
                              ap[f"xk{b}"][:, :, hf * 1024:hf * 1024 + n])
            xkt[hf] = t
            t = self.stream.tile([128, 8, 1024], BF16, tag="xv", name="xvt")
            nc.sync.dma_start(t[:, :, 0:n],
                              ap[f"xv{b}"][:, :, hf * 1024:hf * 1024 + n])
            xvt[hf] = t

        def qgroup(hf, q2):
            def go():
                qb = hf * 2 + q2
                psq = self.psum.tile([128, 512], F32, tag="pqk", bufs=1,
                                     name="psq")
                for dj in range(8):
                    nc.tensor.matmul(psq[:], self.wq[:, dj, :],
                                     xqt[hf][:, dj, q2 * 512:(q2 + 1) * 512],
                                     start=(dj == 0), stop=(dj == 7))
                nc.vector.tensor_copy(
                    self.qT[b][:, qb * 512:(qb + 1) * 512], psq[:])
            return go

        def kgroup(hf, k2, m):
            def go():
                psk = self.psum.tile([128, 512], F32, tag="pqk", bufs=1,
                                     name="psk")
                for dj in range(8):
                    nc.tensor.matmul(psk[:, 0:m], self.wk[:, dj, :],
                                     xkt[hf][:, dj, k2 * 512:k2 * 512 + m],
                                     start=(dj == 0), stop=(dj == 7))
                o = hf * 1024 + k2 * 512
                nc.vector.tensor_copy(self.kT[b][:, o:o + m], psk[:, 0:m])
            return go

        def vgroup(hf, s2s):
            def go():
                for s2 in s2s:
                    sc = hf * 8 + s2
                    psv = self.psum.tile([128, 2, HD], F32, tag="pv", bufs=1,
                                         name="psv")
                    for dj in range(8):
                        nc.tensor.matmul(
                            psv[:], xvt[hf][:, dj, s2 * 128:(s2 + 1) * 128],
                            self.wv[:, dj, :, :],
                            start=(dj == 0), stop=(dj == 7))
                    nc.vector.tensor_copy(self.v[b][sc][:], psv[:])
            return go

        for hf in range(2):
            for q2 in range(2):
                self.fills.append(qgroup(hf, q2))
        for hf in range((nk + 1023) // 1024):
            n = min(1024, nk - hf * 1024)
            for k2 in range((n + 511) // 512):
                self.fills.append(kgroup(hf, k2, min(512, n - k2 * 512)))
            chunks = list(range(n // 128))
            for j in range(0, len(chunks), 2):
                self.fills.append(vgroup(hf, chunks[j:j + 2]))

    def push_o(self, b, qb):
        nc, ap = self.nc, self.ap

        def ochunk(sc):
            def go():
                ot = self.wrk.tile([128, 2, 512], F16, tag="ot", bufs=4,
                                   name="ot")
                for ih in range(2):
                    tg = "pqk" if ih == 0 else "pv"
                    po = self.psum.tile([128, 512], F32, tag=tg, bufs=1,
                                        name="po")
                    nc.tensor.matmul(
                        po[:], self.ctx[b][:, sc * 128:(sc + 1) * 128],
                        self.wo[:, ih * 512:(ih + 1) * 512],
                        start=True, stop=True)
                    nc.vector.tensor_copy(ot[:, ih, :], po[:])
                nc.sync.dma_start(
                    ap["out"][b, sc * 128:(sc + 1) * 128, :], ot[:])
            return go

        for sc in range(4 * qb, 4 * qb + 4):
            self.fills.append(ochunk(sc))

    def pop_fill(self):
        if self.fills:
            self.fills.pop(0)()

    def drain_fills(self):
        while self.fills:
            self.fills.pop(0)()

    # ---- the global attention tile stream ----

    def run(self, order):
        nc = self.nc
        # startup: first batch's projections emitted as a block
        self.push_proj(order[0])
        self.drain_fills()
        pushed = {order[0]}

        pend = None  # (avden, ex, b, kc, first, last, qb)

        def flush():
            nonlocal pend
            if pend is None:
                return
            avden, ex, b, kc, first, last, qb = pend
            pend = None
            ones = self.ones[:, 1 + b, :] if last else self.ones[:, 0, :]
            for h in range(2):
                nc.tensor.matmul(
                    avden[64 * h:64 * h + 64, 0, :],
                    self.v[b][kc][:, h, :], ex[:, h, :],
                    start=first, stop=last)
            for h in range(2):
                nc.tensor.matmul(
                    avden[64 * h:64 * h + 64, 1, :], ones,
                    ex[:, h, :], start=first, stop=last)
            if last:
                recb = self.wrk.tile([128, 512], F32, tag="recb", name="recb")
                nc.vector.reciprocal_approx_fast(recb[:], avden[:, 1, :])
                nc.vector.tensor_mul(
                    self.ctx[b][:, qb * 512:(qb + 1) * 512],
                    avden[:, 0, :], recb[:])
                self.push_o(b, qb)

        for i, b in enumerate(order):
            kcb = self.kcs[b]
            # Prefetch projections for upcoming batches: the next one, plus
            # one more if the next attention segment is too small to host it.
            j = i + 1
            while j < len(order) and order[j] not in pushed:
                self.push_proj(order[j])
                pushed.add(order[j])
                if self.kcs[order[j]] > 2:
                    break
                j += 1
            for qb in range(4):
                avden_cur = self.psum.tile([128, 2, 512], F32, tag="avden",
                                           bufs=1, name="avden")
                for kc in range(kcb):
                    scp = self.psum.tile([128, 2, 512], F32, tag="sc",
                                         bufs=2, name="scp")
                    for h in range(2):
                        nc.tensor.matmul(
                            scp[:, h, :],
                            self.kT[b][64 * h:64 * h + 64,
                                       kc * 128:(kc + 1) * 128],
                            self.qT[b][64 * h:64 * h + 64,
                                       qb * 512:(qb + 1) * 512],
                            start=True, stop=True)
                    ex = self.expool.tile([128, 2, 512], BF16, tag="ex",
                                          name="ex")
                    nc.scalar.activation(ex[:], scp[:], AF.Exp, scale=0.125)
                    flush()
                    pend = (avden_cur, ex, b, kc, kc == 0, kc == kcb - 1, qb)
                    self.pop_fill()
        flush()
        self.drain_fills()
        self.es.close()


def _emit(nc, tc, ap, kcs):
    em = _Emitter(nc, tc, ap, kcs)
    # Largest first (gets the startup shadow), smallest hosted mid-stream,
    # a large batch last so its own O-projection chunks are hidden.
    order = sorted(range(B), key=lambda b: -kcs[b])
    order = order[:-2] + [order[-1], order[-2]]
    em.run(order)


def _build(kcs):
    key = ("nc", tuple(kcs))
    if key in _cache:
        return _cache[key]
    nc = bacc.Bacc("TRN2", target_bir_lowering=False, debug=False,
                   num_devices=8)
    ap = {"wq": nc.dram_tensor("wq", [128, 8, 128], BF16,
                               kind="ExternalInput").ap(),
          "wk": nc.dram_tensor("wk", [128, 8, 128], BF16,
                               kind="ExternalInput").ap(),
          "wv": nc.dram_tensor("wv", [128, 8, 2, HD], BF16,
                               kind="ExternalInput").ap(),
          "wo": nc.dram_tensor("wo", [128, D], BF16,
                               kind="ExternalInput").ap(),
          "ones": nc.dram_tensor("ones", [128, 5, 64], BF16,
                                 kind="ExternalInput").ap(),
          "out": nc.dram_tensor("out", [B, S, D], F16,
                                kind="ExternalOutput").ap()}
    for b in range(B):
        ap[f"xq{b}"] = nc.dram_tensor(f"xq{b}", [128, 8, S], BF16,
                                      kind="ExternalInput").ap()
        ap[f"xk{b}"] = nc.dram_tensor(f"xk{b}", [128, 8, kcs[b] * 128], BF16,
                                      kind="ExternalInput").ap()
        ap[f"xv{b}"] = nc.dram_tensor(f"xv{b}", [128, 8, kcs[b] * 128], BF16,
                                      kind="ExternalInput").ap()
    with tile.TileContext(nc) as tc:
        _emit(nc, tc, ap, kcs)
    nc.compile()
    _cache[key] = nc
    return nc


def _blocked(x2d):
    """[Dsub, N] -> [128, Dsub//128, N] blocked layout (partition, dj, col)."""
    d, n = x2d.shape
    return np.ascontiguousarray(
        x2d.reshape(d // 128, 128, n).transpose(1, 0, 2))


def _in_maps(kcs, queries, keys, values, valid_len, W_q, W_k, W_v, W_o):
    bf = ml_dtypes.bfloat16
    shared = {}
    for b in range(B):
        nk = kcs[b] * 128
        xv = values[b][:nk].T.copy()      # [D, nk]
        xv[:, int(valid_len[b]):] = 0.0   # mask padding rows of v
        shared[f"xq{b}"] = _blocked(queries[b].T.astype(bf))
        shared[f"xk{b}"] = _blocked(keys[b][:nk].T.astype(bf))
        shared[f"xv{b}"] = _blocked(xv.astype(bf))
    ones = np.zeros((128, 5, 64), bf)
    ones[:, 0, :] = 1.0
    p = np.arange(128)
    for b in range(B):
        valid = ((kcs[b] - 1) * 128 + p < int(valid_len[b])).astype(bf)
        ones[:, 1 + b, :] = valid[:, None]
    shared["ones"] = ones

    maps = []
    for c in range(8):
        j0 = 128 * c
        m = dict(shared)
        m["wq"] = _blocked(
            np.ascontiguousarray(W_q[j0:j0 + 128, :].T).astype(bf))
        m["wk"] = _blocked(
            np.ascontiguousarray(W_k[j0:j0 + 128, :].T).astype(bf))
        m["wv"] = _blocked(
            np.ascontiguousarray(W_v[j0:j0 + 128, :].T).astype(bf)
        ).reshape(128, 8, 2, HD)
        m["wo"] = np.ascontiguousarray(W_o[:, j0:j0 + 128].T).astype(bf)
        maps.append(m)
    return maps


def kernel(queries, keys, values, valid_len, W_q, W_k, W_v, W_o,
           _run_kwargs=None):
    queries = np.asarray(queries, np.float32)
    keys = np.asarray(keys, np.float32)
    values = np.asarray(values, np.float32)
    valid_len = np.asarray(valid_len)
    W_q = np.asarray(W_q, np.float32)
    W_k = np.asarray(W_k, np.float32)
    W_v = np.asarray(W_v, np.float32)
    W_o = np.asarray(W_o, np.float32)

    kcs = [max(1, min(16, -(-int(valid_len[b]) // 128))) for b in range(B)]
    nc = _build(kcs)
    maps = _in_maps(kcs, queries, keys, values, valid_len, W_q, W_k, W_v, W_o)
    res = run_bass_kernel_spmd(nc, maps, list(range(8)), **(_run_kwargs or {}))
    out = np.zeros((B, S, D), np.float32)
    for c in range(8):
        out += res.results[c]["out"].astype(np.float32)
    if _run_kwargs:
        _cache["last_results"] = res
    return out
